# revision 1
# baseline (speedup 1.0000x reference)
"""Chamfer loss kernel for Trainium2 (8 NeuronCores, data-parallel over batch).

loss = 0.5 * (sum_n min_m ||x_n - y_m||^2 + sum_m min_n ||x_n - y_m||^2)

Strategy per core (2 batches of the 16):
  - Build augmented operands W_x = [-2x^T; ones; x2] (K=66 rows) and
    W_y = [y^T; y2; ones] so a single f32r matmul tile (1 cyc/row vs 4 for
    plain fp32) directly yields dist[n,m] = x2[n] + y2[m] - 2 x.y in PSUM.
  - ScalarE casts each PSUM tile to fp16 in SBUF (ScalarE is the only other
    engine besides VectorE with a PSUM port, and it cannot do mins).
  - VectorE does every min in fp16 2x_1P mode (HW-measured 1.07us per
    [128,2048] tensor_tensor vs 2.13us for tensor_reduce, which measures
    1x): column mins via a running elementwise-min accumulator; row mins
    via an elementwise min of the two row chunks + an in-place strided
    min-tree down to 128 wide, collected per batch and finished by a
    single segmented reduce.
  - Column accumulators are finalized with PE transposes + a segmented
    free-axis reduce; clamp at 0 after the mins (max(.,0) is monotone so
    this equals clamping before), sum on chip to one scalar per core, and
    sum the 8 core scalars on the host.
"""

import sys

sys.path.insert(0, "/opt/trn_rl_repo")

import numpy as np

B, N, M, D = 16, 4096, 4096, 64
NCORES = 8
BPC = B // NCORES  # batches per core
NB = N // 128      # n blocks (128 rows each)
MCW = 2048         # m chunk width (4 psum banks)
NMC = M // MCW     # m chunks
NMM = MCW // 512   # matmuls per chunk
K = D + 2          # augmented contraction dim

_cached = None


def _build(reps=1):
    import concourse.bacc as bacc
    import concourse.tile as tile
    from concourse import mybir

    f32 = mybir.dt.float32
    f32r = mybir.dt.float32r
    f16 = mybir.dt.float16
    AX = mybir.AxisListType.X
    MIN = mybir.AluOpType.min
    Copy = mybir.ActivationFunctionType.Copy
    Square = mybir.ActivationFunctionType.Square

    nc = bacc.Bacc(
        "TRN2",
        target_bir_lowering=False,
        debug=False,
        enable_asserts=False,
        num_devices=NCORES,
    )

    xm2_d = nc.dram_tensor("xm2", [BPC, N, D], f32, kind="ExternalInput")
    y_d = nc.dram_tensor("y", [BPC, M, D], f32, kind="ExternalInput")
    loss_d = nc.dram_tensor("loss", [1, 1], f32, kind="ExternalOutput")
    id32_d = nc.inline_tensor(np.eye(128, dtype=np.float32), name="id32")
    ones_d = nc.inline_tensor(np.ones((1, N), dtype=np.float32), name="ones_row")

    with tile.TileContext(nc) as tc:
        with (
            tc.tile_pool(name="psum", bufs=2, space="PSUM") as psp,
            tc.tile_pool(name="wts", bufs=2) as wpool,
            tc.tile_pool(name="inb", bufs=2) as inpool,
            tc.tile_pool(name="sq", bufs=2) as sqpool,
            tc.tile_pool(name="dist", bufs=4) as dpool,
            tc.tile_pool(name="acc", bufs=2) as apool,
            tc.tile_pool(name="small", bufs=4) as spool,
            tc.tile_pool(name="fin", bufs=1) as fpool,
        ):
            halfcol = fpool.tile([128, 1], f32, tag="halfcol")
            nc.gpsimd.memset(halfcol[:], 0.5)
            id32t = fpool.tile([128, 128], f32, tag="id32")
            nc.sync.dma_start(out=id32t[:], in_=id32_d.ap())
            id32 = id32t[:]
            # per-(batch,direction) partition-wise partial sums
            contribs = fpool.tile([128, 2 * BPC], f32, tag="contribs")

            def setup(b):
                # load inputs, build W_x [K,4096], W_y [K,4096].  The two
                # halves of each load go to different engines' HWDGE queues so
                # the four 1MB transfers run concurrently.
                # Contiguous loads: partition p takes 32 consecutive points
                # (8KB per partition -> full DMA bandwidth). This permutes the
                # point order (n = p*32 + r), which the loss is invariant to;
                # the same xbig/ybig layout feeds both the transposes and the
                # norm rows, so the permutation stays consistent.
                engs = [nc.sync, nc.scalar, nc.gpsimd, nc.sync]
                xbig = inpool.tile([128, NB, D], f32, tag="xb", name=f"xbig_{b}")
                xsrc = xm2_d.ap()[b].rearrange("(p a) k -> p a k", p=128)
                ybig = inpool.tile([128, NB, D], f32, tag="yb", name=f"ybig_{b}")
                ysrc = y_d.ap()[b].rearrange("(p a) k -> p a k", p=128)
                engs[2 * b].dma_start(out=xbig[:], in_=xsrc)
                engs[2 * b + 1].dma_start(out=ybig[:], in_=ysrc)

                wx = wpool.tile([K, N], f32r, tag="wx", name=f"wx_{b}")
                wy = wpool.tile([K, M], f32r, tag="wy", name=f"wy_{b}")
                nc.sync.dma_start(out=wx[D : D + 1, :], in_=ones_d.ap().bitcast(f32r))
                nc.sync.dma_start(out=wy[D + 1 : D + 2, :], in_=ones_d.ap().bitcast(f32r))

                # transpose inputs into W rows 0:64 (PE transpose + ACT copyback)
                for src_, w in ((ybig, wy), (xbig, wx)):
                    for g in range(NB // 8):
                        sp = psp.tile([D, MCW // 2], f32, tag="big", name=f"sp_{b}_{g}")
                        for j in range(8):
                            nc.tensor.transpose(
                                sp[:, j * 128 : (j + 1) * 128],
                                src_[:, g * 8 + j, :],
                                id32,
                            )
                        nc.scalar.activation(
                            w[0:D, g * (MCW // 2) : (g + 1) * (MCW // 2)], sp[:], Copy
                        )

                # norm rows in free layout, from untransposed inputs:
                # wx row 65 = x2 = sum((0.5*xm2)^2); wy row 64 = y2 = sum(y^2).
                # square+rowsum in partition layout, one PE transpose, then a
                # partition->free scatter DMA into the single W row.
                for src_, w, scl, row in (
                    (ybig, wy, 1.0, D),
                    (xbig, wx, 0.5, D + 1),
                ):
                    sqb = sqpool.tile([128, NB * D], f32, tag="sq", name=f"sq_{b}_{row}")
                    nc.scalar.activation(
                        sqb[:], src_[:].rearrange("p a k -> p (a k)"), Square, scale=scl
                    )
                    s2pl = spool.tile([128, NB], f32, tag="s2pl", bufs=2)
                    nc.vector.tensor_reduce(
                        s2pl[:],
                        sqb[:].rearrange("p (a k) -> p a k", k=D),
                        AX,
                        mybir.AluOpType.add,
                    )
                    s2T = psp.tile([NB, 128], f32, tag="big", name=f"s2T_{b}_{row}")
                    nc.tensor.transpose(s2T[:], s2pl[:], id32)
                    stage = spool.tile([NB, 128], f32, tag="stage", bufs=2)
                    nc.scalar.activation(stage[:], s2T[:], Copy)
                    nc.sync.dma_start(
                        out=w[row : row + 1, :], in_=stage[:].bitcast(f32r)
                    )

                return wx, wy

            def main(b, wx, wy, mid_hook=None):
                # distance tiles, row mins, column-min accumulators
                acc = apool.tile([128, NMC * MCW], f16, tag="acc", name=f"acc_{b}")
                inited = [False]
                rowall = spool.tile([128, NB], f32, tag="rowall", bufs=2)
                colall = spool.tile([128, NB], f32, tag="colall", bufs=2)
                rowtree = spool.tile(
                    [128, NB * 128], f16, tag="rowtree", bufs=2, name=f"rowtree_{b}"
                )

                seq = [i for _ in range(reps) for i in range(NB)]
                for pos, nb in enumerate(seq):
                    if pos == 16 and mid_hook is not None:
                        mid_hook()
                    # Row mins: min the two chunks, then an in-place strided
                    # min-tree (all fp16 tensor_tensor at 2x) down to 256 wide
                    # before one short 1x reduce.
                    first = nb == 0 and not inited[0]
                    if first:
                        inited[0] = True
                        dist = acc
                    else:
                        dist = dpool.tile(
                            [128, NMC * MCW], f16, tag="dist", name=f"dist_{b}_{nb}"
                        )
                    for mc in range(NMC):
                        pt = psp.tile([128, MCW], f32, tag="big", name=f"pt_{b}_{nb}_{mc}")
                        for j in range(NMM):
                            nc.tensor.matmul(
                                pt[:, j * 512 : (j + 1) * 512],
                                wx[:, nb * 128 : (nb + 1) * 128],
                                wy[:, mc * MCW + j * 512 : mc * MCW + (j + 1) * 512],
                                start=True,
                                stop=True,
                            )
                        nc.scalar.activation(
                            dist[:, mc * MCW : (mc + 1) * MCW], pt[:], Copy
                        )
                    if not first:
                        nc.vector.tensor_tensor(acc[:], acc[:], dist[:], MIN)
                    racc = dpool.tile([128, MCW], f16, tag="racc", bufs=3)
                    nc.vector.tensor_tensor(
                        racc[:], dist[:, MCW : 2 * MCW], dist[:, 0:MCW], MIN
                    )
                    w_ = MCW // 2
                    while w_ >= 256:
                        nc.vector.tensor_tensor(
                            racc[:, 0:w_], racc[:, 0:w_], racc[:, w_ : 2 * w_], MIN
                        )
                        w_ //= 2
                    nc.vector.tensor_tensor(
                        rowtree[:, nb * 128 : (nb + 1) * 128],
                        racc[:, 0:128],
                        racc[:, 128:256],
                        MIN,
                    )

                nc.vector.tensor_reduce(
                    rowall[:],
                    rowtree[:].rearrange("p (a c) -> p a c", c=128),
                    AX,
                    MIN,
                )

                # finalize column mins: transpose accumulators, segmented reduce
                for mc in range(NMC):
                    acc32 = sqpool.tile([128, MCW], f32, tag="acc32", bufs=2)
                    nc.scalar.activation(
                        acc32[:], acc[:, mc * MCW : (mc + 1) * MCW], Copy
                    )
                    ptT = psp.tile([128, MCW], f32, tag="big", name=f"ptT_{b}_{mc}")
                    for t in range(MCW // 128):
                        nc.tensor.transpose(
                            ptT[:, t * 128 : (t + 1) * 128],
                            acc32[:, t * 128 : (t + 1) * 128],
                            id32,
                        )
                    nc.vector.tensor_reduce(
                        colall[:, mc * 16 : (mc + 1) * 16],
                        ptT[:].rearrange("p (t c) -> p t c", c=128),
                        AX,
                        MIN,
                    )

                # clamp then per-partition sums
                for i, mins in enumerate((rowall, colall)):
                    rl = spool.tile([128, NB], f32, tag="rl", bufs=2)
                    nc.vector.tensor_scalar_max(rl[:], mins[:], 0.0)
                    nc.vector.reduce_sum(
                        contribs[:, 2 * b + i : 2 * b + i + 1], rl[:], axis=AX
                    )

            # setup(1) is emitted a few row-blocks into main(0) so its
            # ScalarE/psum work overlaps the main stream instead of
            # lengthening the prologue.
            w0 = setup(0)
            later = {}

            def hook():
                later["w1"] = setup(1)

            main(0, *w0, mid_hook=hook)
            main(1, *later["w1"])

            # ---- final: 0.5 * total over partitions and contributions ----
            fin = psp.tile([1, 2 * BPC], f32, tag="big")
            nc.tensor.matmul(
                fin[:], halfcol[:], contribs[:], start=True, stop=True
            )
            finsb = fpool.tile([1, 1], f32, tag="finsb")
            nc.vector.reduce_sum(finsb[:], fin[:], axis=AX)
            nc.sync.dma_start(out=loss_d.ap(), in_=finsb[:])

    nc.compile()
    return nc


def _get_nc():
    global _cached
    if _cached is None:
        _cached = _build()
    return _cached


def _in_maps(x, y):
    x = np.ascontiguousarray(np.asarray(x, dtype=np.float32))
    y = np.ascontiguousarray(np.asarray(y, dtype=np.float32))
    maps = []
    for c in range(NCORES):
        sl = slice(c * BPC, (c + 1) * BPC)
        maps.append({"xm2": -2.0 * x[sl], "y": y[sl]})
    return maps


def _run(x, y, trace=False):
    from concourse.bass_utils import run_bass_kernel_spmd

    nc = _get_nc()
    res = run_bass_kernel_spmd(
        nc, _in_maps(x, y), list(range(NCORES)), trace=trace
    )
    total = sum(float(r["loss"][0, 0]) for r in res.results)
    return np.array(total, dtype=np.float32), res


def kernel(x, y):
    out, _ = _run(x, y)
    return out


if __name__ == "__main__":
    rng = np.random.default_rng(0)
    x = rng.standard_normal((B, N, D)).astype(np.float32)
    y = rng.standard_normal((B, M, D)).astype(np.float32)
    got = kernel(x, y)
    x2 = (x * x).sum(-1)
    y2 = (y * y).sum(-1)
    xy = np.einsum("bnd,bmd->bnm", x, y, optimize=True)
    dist = np.maximum(x2[:, :, None] + y2[:, None, :] - 2.0 * xy, 0.0)
    want = dist.min(-1).sum() * 0.5 + dist.min(-2).sum() * 0.5
    print("got", got, "want", want, "rel", abs(got - want) / abs(want))

